# revision 28
# baseline (speedup 1.0000x reference)
"""Multi-head self-attention TRN2 Bass kernel (v2: PE-saturated schedule).

Problem: x[2, 2048, 1024], 16 heads x 64 dim, fp32.
Sharding: 8 cores = 2 batches x 4 head-groups (4 heads each).
Each core computes its batch's partial output (its 4 heads through
QKV -> attention -> output projection rows); host sums the 4 partials
per batch and adds bo.

The PE (tensor engine) is the bottleneck (~167us of streamed matmul
at bf16: scores 54.6 + attn@V 54.6 + QKV 41 + out_proj 13.6); the
ACT exp stream is ~142us. v2 therefore schedules for zero PE idle:

  - minimal warmup: only kT m=0 chunk 0 (no bias -- bk is dropped
    entirely: it shifts scores by a per-q constant, softmax-invariant),
    v chunks 0-3, qT m=0 chunk 0. Everything else streams into the
    attention sweep as dependency-placed inline fillers.
  - exp activation table pre-loaded during the initial DMA wait;
    dummy matmuls on a memset tile keep the PE p-state warm while
    the first x chunk lands.
  - scores->AV lag of 2 iterations so the in-order PE queue never
    head-stalls on the ACT stream.
  - normalize (evac + reciprocal + rank-1 PE broadcast) of pair p is
    deferred into pair p+1's early iterations; out_proj of q-chunk qc
    runs inside (qc+1, mt=0)'s sweep.
"""

import numpy as np

S = 2048          # sequence length per batch
H = 1024          # hidden
G = 256           # head-group width (4 heads x 64)
HD = 65           # V' columns per head (64 + ones)
NHL = 4           # heads per core
N_CORES = 8

_CACHE = {}


def _build():
    if "nc" in _CACHE:
        return _CACHE["nc"]

    import concourse.bass as bass
    import concourse.mybir as mybir
    import concourse.tile as tile
    from concourse import bacc
    from concourse.tile_rust import add_dep_helper

    f32 = mybir.dt.float32
    bf16 = mybir.dt.bfloat16
    f8 = mybir.dt.float8e4
    EXP = mybir.ActivationFunctionType.Exp
    DR = mybir.MatmulPerfMode.DoubleRow

    nc = bacc.Bacc("TRN2", target_bir_lowering=False, debug=False,
                   num_devices=N_CORES)

    xt_in = nc.dram_tensor("xt", [4, 128, 8, 512], bf16, kind="ExternalInput")
    wq_in = nc.dram_tensor("wq", [H, G], bf16, kind="ExternalInput")
    wk_in = nc.dram_tensor("wk", [H, G], bf16, kind="ExternalInput")
    wv_in = nc.dram_tensor("wv", [H, G], bf16, kind="ExternalInput")
    bq_in = nc.dram_tensor("bq", [G, 1], f32, kind="ExternalInput")
    bv_in = nc.dram_tensor("bv", [G], f32, kind="ExternalInput")
    wo_in = nc.dram_tensor("wo", [NHL, 64, H], bf16, kind="ExternalInput")
    out_d = nc.dram_tensor("out", [S, H], bf16, kind="ExternalOutput")

    with tile.TileContext(nc) as tc:
        with tc.tile_pool(name="persist", bufs=1) as persist:
            qT = persist.tile([128, 2, S], bf16)     # [qd, m, s]
            kT = persist.tile([128, 2, S], bf16)
            vp = persist.tile([128, 16, NHL, HD], bf16)  # [s-part, st, h, col]
            bq_sb = persist.tile([128, 2, 1], f32)
            bv_f = persist.tile([1, G], f32)
            bv_bf = persist.tile([1, G], bf16)
            bv_bc = persist.tile([128, G], f32)
            wo_pr = persist.tile([128, 2, H], bf16)
            ones64 = persist.tile([1, 64], bf16)
            ones128 = persist.tile([1, 128], bf16)
            warm = persist.tile([128, 512], bf16)
            warm_e = persist.tile([1, 8], f32)
            wq_sb = persist.tile([128, 8, G], bf16)
            wk_sb = persist.tile([128, 8, G], bf16)
            wv_sb = persist.tile([128, 8, G], bf16)
            xTc = [persist.tile([128, 8, 512], bf16, name=f"xT_{jc}")
                   for jc in range(4)]

            with (
                tc.tile_pool(name="at_roll", bufs=2) as at_pool,
                tc.tile_pool(name="outP", bufs=4) as op_pool,
                tc.tile_pool(name="tmpo", bufs=1) as tmpo_pool,
                tc.tile_pool(name="sums", bufs=4) as sums_pool,
                tc.tile_pool(name="osb", bufs=2) as osb_pool,
                tc.tile_pool(name="ps_s", bufs=2, space="PSUM") as ps_s_pool,
                tc.tile_pool(name="ps_av", bufs=2, space="PSUM") as ps_av_pool,
                tc.tile_pool(name="ps_op", bufs=1, space="PSUM") as ps_op_pool,
            ):
                # memsets first so the gpsimd queue isn't blocked
                nc.gpsimd.memset(warm, 0.125)
                nc.gpsimd.memset(vp[:, :, :, 64:65], 1.0)
                nc.gpsimd.memset(ones64, 1.0)
                nc.gpsimd.memset(ones128, 1.0)

                # ---------------- DMAs (two priority chains) ----------------
                # chain A (small): bv -> wk -> wv -> wq -> bq
                dma_bv = nc.sync.dma_start(
                    out=bv_f, in_=bv_in.ap().rearrange("(o g) -> o g", o=1))
                dma_wk = nc.sync.dma_start(
                    out=wk_sb, in_=wk_in.ap().rearrange("(t p) d -> p t d", p=128))
                dma_wv = nc.sync.dma_start(
                    out=wv_sb, in_=wv_in.ap().rearrange("(t p) d -> p t d", p=128))
                dma_wq = nc.sync.dma_start(
                    out=wq_sb, in_=wq_in.ap().rearrange("(t p) d -> p t d", p=128))
                dma_bq = nc.sync.dma_start(
                    out=bq_sb, in_=bq_in.ap().rearrange("(m p) o -> p m o", p=128))
                # chain B (big): xc0 (split in halves so the warmup matmuls
                # can start after 512KB) -> xc1 -> xc2 -> xc3 -> wo
                x0a = nc.sync.dma_start(out=xTc[0][:, 0:4, :],
                                        in_=xt_in.ap()[0][:, 0:4, :])
                x0b = nc.sync.dma_start(out=xTc[0][:, 4:8, :],
                                        in_=xt_in.ap()[0][:, 4:8, :])
                x_dmas = [x0b] + [nc.sync.dma_start(out=xTc[jc],
                                                    in_=xt_in.ap()[jc])
                          for jc in range(1, 4)]
                # Wo as stacked head pairs: [two*64+p, pr, n]
                dma_wo = nc.sync.dma_start(
                    out=wo_pr,
                    in_=wo_in.ap().rearrange("(pr two) p n -> (two p) pr n", two=2))
                add_dep_helper(x0b.ins, x0a.ins, reason="dma order")
                for a, b in [(dma_wk, dma_bv), (dma_wv, dma_wk),
                             (dma_wq, dma_wv), (dma_bq, dma_wq),
                             (x_dmas[1], x_dmas[0]), (x_dmas[2], x_dmas[1]),
                             (x_dmas[3], x_dmas[2]), (dma_wo, x_dmas[3])]:
                    add_dep_helper(a.ins, b.ins, reason="dma order")

                # pre-load the exp activation table (~2.7us) off the
                # critical path
                nc.scalar.activation(out=warm_e, in_=warm[0:1, 0:8], func=EXP)

                def dummy(n=512):
                    ps_d = ps_op_pool.tile([128, 512], f32, tag="dummy",
                                           bufs=1)
                    nc.tensor.matmul(ps_d[:, 0:n], lhsT=warm[:, 0:128],
                                     rhs=warm[:, 0:n], start=True, stop=True)

                for _ in range(2):
                    dummy()
                # bv broadcast along partitions via rank-1 PE outer product
                nc.vector.tensor_copy(bv_bf, bv_f)
                ps_bv = ps_op_pool.tile([128, G], f32, tag="oproj",
                                        name="ps_bv", bufs=1)
                nc.tensor.matmul(ps_bv, lhsT=ones128, rhs=bv_bf,
                                 start=True, stop=True)
                nc.vector.tensor_copy(bv_bc, ps_bv)
                for _ in range(3):
                    dummy()

                # ---------------- QKV building blocks ----------------
                qk_ring = [0]

                def qk_full(w_sb, b_sb, dst, jc, m):
                    """dst[:, m, jc-chunk] = (x @ W)[:, m-half] (+ bias).

                    PSUM comes from the two 1-buf rings (dummy/oproj)
                    alternately so adjacent calls don't WAR-stall."""
                    sl = slice(jc * 512, (jc + 1) * 512)
                    tag = "dummy" if qk_ring[0] == 0 else "oproj"
                    qk_ring[0] ^= 1
                    ps = ps_op_pool.tile([128, 512], f32, tag=tag,
                                         name=f"psqk_{id(w_sb)}_{jc}_{m}",
                                         bufs=1)
                    for ht in range(8):
                        nc.tensor.matmul(
                            ps,
                            lhsT=w_sb[:, ht, m * 128:(m + 1) * 128],
                            rhs=xTc[jc][:, ht, :],
                            start=(ht == 0), stop=(ht == 7))
                    if b_sb is not None:
                        nc.vector.tensor_scalar_add(dst[:, m, sl], ps,
                                                    b_sb[:, m, :])
                    else:
                        nc.vector.tensor_copy(dst[:, m, sl], ps)

                def v_unit(st16):
                    tag = "dummy" if qk_ring[0] == 0 else "oproj"
                    qk_ring[0] ^= 1
                    ps_vt = ps_op_pool.tile([128, 512], f32, tag=tag,
                                            name=f"psv_{st16}", bufs=1)
                    for ht in range(8):
                        nc.tensor.matmul(
                            ps_vt[:, 0:G],
                            lhsT=xTc[st16 // 4][:, ht,
                                                (st16 % 4) * 128:
                                                (st16 % 4 + 1) * 128],
                            rhs=wv_sb[:, ht, :],
                            start=(ht == 0), stop=(ht == 7))
                    nc.vector.tensor_add(
                        vp[:, st16, :, 0:64],
                        ps_vt[:, 0:G].rearrange("p (h d) -> p h d", h=NHL),
                        bv_bc.rearrange("p (h d) -> p h d", h=NHL))

                # warmup: exactly what pair 0 iterations 0-3 need
                qk_full(wk_sb, None, kT, 0, 0)
                for i in range(4):
                    v_unit(i)
                qk_full(wq_sb, bq_sb, qT, 0, 0)

                # ---------------- normalize + out_proj ----------------
                def norm_evac(ps_av, hh, tag):
                    # evacuate PSUM right away to release the bank; MUST be
                    # emitted before the next pair's first AV matmul so the
                    # ring WAR dependency is seen
                    uout = tmpo_pool.tile([HD, 512], f32, tag="uout",
                                          name=f"uo_{tag}_{hh}", bufs=4)
                    nc.vector.tensor_copy(uout, ps_av)
                    return uout

                def norm_recip(uout, hh, tag):
                    # DVE chain: sums copy -> fast reciprocal -> bf16 cast
                    sums = sums_pool.tile([1, 512], f32, tag="sums",
                                          name=f"sm_{tag}_{hh}")
                    nc.vector.tensor_copy(sums, uout[64:65, :])
                    recip = sums_pool.tile([1, 512], f32, tag="recip",
                                           name=f"rc_{tag}_{hh}")
                    nc.vector.reciprocal_approx_fast(out=recip, in_=sums)
                    recip_bf = sums_pool.tile([1, 512], bf16, tag="recipb",
                                              name=f"rcb_{tag}_{hh}")
                    nc.vector.tensor_copy(recip_bf, recip)
                    return recip_bf

                def norm_fin(outP, uout, recip_bf, hh, tag, tail=False):
                    if tail:
                        # at the kernel tail the PE is idle and the gpsimd
                        # broadcast's ~1us latency is serial: use the rank-1
                        # PE outer product instead
                        rbc_ps = ps_op_pool.tile([64, 512], f32, tag="dummy",
                                                 name=f"rbp_{tag}_{hh}",
                                                 bufs=1)
                        nc.tensor.matmul(rbc_ps, lhsT=ones64, rhs=recip_bf,
                                         start=True, stop=True)
                        nc.vector.tensor_mul(
                            outP[hh * 64:hh * 64 + 64, :], uout[0:64, :],
                            rbc_ps)
                        return
                    # broadcast along partitions on the idle GPSIMD engine
                    # (keeps the PE out of the normalize chain entirely)
                    rbc = sums_pool.tile([64, 512], bf16, tag="rbc",
                                         name=f"rb_{tag}_{hh}")
                    nc.gpsimd.partition_broadcast(rbc, recip_bf)
                    nc.vector.tensor_mul(
                        outP[hh * 64:hh * 64 + 64, :], uout[0:64, :], rbc)

                def oproj_unit(qc, outPs, qt, tail=False):
                    # out_proj for one q-tile (K=128 stacked pairs); the two
                    # ncx halves alternate the 1-buf rings so ncx1's matmuls
                    # don't WAR-stall on ncx0's evacuation; at the kernel
                    # tail the freed score slots double-buffer it
                    osb = osb_pool.tile([128, H], bf16, tag="osb",
                                        name=f"osb_{qc}_{qt}")
                    for ncx in range(2):
                        if tail:
                            ps_op = ps_s_pool.tile(
                                [128, 2, 512], f32, tag="s",
                                name=f"psot_{qc}_{qt}_{ncx}")[:, 0, :]
                        else:
                            ps_op = ps_op_pool.tile(
                                [128, 512], f32,
                                tag="oproj" if ncx == 0 else "dummy",
                                name=f"pso_{qc}_{qt}_{ncx}", bufs=1)
                        for pr in range(2):
                            nc.tensor.matmul(
                                ps_op,
                                lhsT=outPs[pr][:, qt * 128:(qt + 1) * 128],
                                rhs=wo_pr[:, pr, ncx * 512:(ncx + 1) * 512],
                                start=(pr == 0), stop=(pr == 1))
                        if tail and ncx == 1:
                            # ACT is idle after the last exp -- split the
                            # tail evacuations across ACT and DVE
                            nc.scalar.copy(
                                osb[:, ncx * 512:(ncx + 1) * 512], ps_op)
                        else:
                            nc.vector.tensor_copy(
                                osb[:, ncx * 512:(ncx + 1) * 512], ps_op)
                    nc.sync.dma_start(
                        out=out_d.ap()[qc * 512 + qt * 128:
                                       qc * 512 + (qt + 1) * 128, :],
                        in_=osb)

                # ---------------- attention sweep ----------------
                pending_norm = None   # (outP, ps_avs, [uouts], [recips], tag)
                prev_oproj = None     # (qc, outPs) awaiting out_proj
                pair_idx = 0
                for qc in range(4):  # q-chunks of 512
                    qsl = slice(qc * 512, (qc + 1) * 512)
                    outPs = {}
                    # qc3 runs mt1 first so the kernel tail only carries
                    # mt0's normalize
                    for mt in ((1, 0) if qc == 3 else (0, 1)):
                        tag = f"{qc}_{mt}"
                        attnT = at_pool.tile([128, 2, 4, 512], bf16,
                                             tag="at", name=f"at_{tag}")
                        ps_avs = [ps_av_pool.tile([HD, 512], f32, tag="av",
                                                  name=f"av_{tag}_{hh}")
                                  for hh in range(2)]

                        def av_mm(kt, ps_avs=ps_avs, attnT=attnT, mt=mt):
                            for hh in range(2):
                                nc.tensor.matmul(
                                    ps_avs[hh],
                                    lhsT=vp[:, kt, 2 * mt + hh, :],
                                    rhs=attnT[:, hh, kt % 4, :],
                                    start=(kt == 0), stop=(kt == 15))

                        for kt in range(16):
                            # inline QKV fillers, placed just before need
                            if pair_idx == 0:
                                if kt % 4 == 0 and kt > 0:
                                    qk_full(wk_sb, None, kT, kt // 4, 0)
                                if 2 <= kt <= 13:
                                    v_unit(kt + 2)
                            elif pair_idx == 1 and kt % 4 == 0:
                                if kt == 0:
                                    qk_full(wq_sb, bq_sb, qT, 0, 1)
                                qk_full(wk_sb, None, kT, kt // 4, 1)
                            elif pair_idx >= 2 and kt == 0:
                                qk_full(wq_sb, bq_sb, qT, qc, mt)
                            # deferred normalize of the previous pair:
                            # evacuations at kt 0/1 (before av_mm(0) below),
                            # DVE recip chains at kt 2/3, PE bcast+mul at 4/6
                            if pending_norm is not None:
                                pP, pavs, puo, prc, ptag = pending_norm
                                if kt in (0, 1):
                                    puo.append(norm_evac(pavs[kt], kt, ptag))
                                elif kt in (2, 3):
                                    prc.append(norm_recip(puo[kt - 2],
                                                          kt - 2, ptag))
                                elif kt == 4:
                                    norm_fin(pP, puo[0], prc[0], 0, ptag)
                                elif kt == 6:
                                    norm_fin(pP, puo[1], prc[1], 1, ptag)
                                    pending_norm = None
                            # deferred out_proj of the previous q-chunk
                            if prev_oproj is not None and kt in (10, 13):
                                pq, pouts = prev_oproj
                                qt0 = 0 if kt == 10 else 2
                                oproj_unit(pq, pouts, qt0)
                                oproj_unit(pq, pouts, qt0 + 1)
                                if kt == 13:
                                    prev_oproj = None
                            # scores (transposed): S^T[k,q] = kT.T @ qT
                            ps_s = ps_s_pool.tile([128, 2, 512], f32, tag="s")
                            for hh in range(2):
                                nc.tensor.matmul(
                                    ps_s[:, hh, :],
                                    lhsT=kT[hh * 64:hh * 64 + 64, mt,
                                            kt * 128:(kt + 1) * 128],
                                    rhs=qT[hh * 64:hh * 64 + 64, mt, qsl],
                                    start=True, stop=True)
                            nc.scalar.activation(
                                out=attnT[:, :, kt % 4, :], in_=ps_s, func=EXP)
                            if kt >= 2:
                                av_mm(kt - 2)
                        av_mm(14)
                        av_mm(15)

                        outP = op_pool.tile([128, 512], bf16, tag="outP",
                                            name=f"outP_{tag}")
                        pending_norm = (outP, ps_avs, [], [], tag)
                        outPs[mt] = outP
                        pair_idx += 1
                    prev_oproj = (qc, [outPs[0], outPs[1]])

                # tail: last pair's normalize, then final out_proj; the
                # uout evacuations ride the idle ACT engine
                pP, pavs, puo, prc, ptag = pending_norm
                for hh in range(2):
                    uout = tmpo_pool.tile([HD, 512], f32, tag="uout",
                                          name=f"uo_{ptag}_{hh}", bufs=4)
                    nc.scalar.copy(uout, pavs[hh])
                    puo.append(uout)
                for hh in range(2):
                    prc.append(norm_recip(puo[hh], hh, ptag))
                for hh in range(2):
                    norm_fin(pP, puo[hh], prc[hh], hh, ptag, tail=True)
                pq, pouts = prev_oproj
                for qt in range(4):
                    oproj_unit(pq, pouts, qt, tail=True)

    nc.compile()
    _CACHE["nc"] = nc
    return nc


def make_in_maps(x, Wq, bq, Wk, bk, Wv, bv, Wo):
    import ml_dtypes
    bf = ml_dtypes.bfloat16

    x = np.asarray(x, dtype=np.float32)
    Wq = np.asarray(Wq, dtype=np.float32)
    bq = np.asarray(bq, dtype=np.float32)
    Wk = np.asarray(Wk, dtype=np.float32)
    Wv = np.asarray(Wv, dtype=np.float32)
    bv = np.asarray(bv, dtype=np.float32)
    Wo = np.asarray(Wo, dtype=np.float32)

    scale = np.float32(1.0 / 8.0)  # 1/sqrt(64)

    in_maps = []
    for core in range(N_CORES):
        b = core // 4
        g = core % 4
        cs = slice(g * G, (g + 1) * G)
        in_maps.append({
            "xt": np.ascontiguousarray(
                x[b].reshape(4, 512, 8, 128).transpose(0, 3, 2, 1)).astype(bf),
            "wq": np.ascontiguousarray(Wq[:, cs] * scale).astype(bf),
            "wk": np.ascontiguousarray(Wk[:, cs]).astype(bf),
            "wv": np.ascontiguousarray(Wv[:, cs]).astype(bf),
            "bq": np.ascontiguousarray((bq[cs] * scale).reshape(G, 1)),
            "bv": np.ascontiguousarray(bv[cs]),
            "wo": np.ascontiguousarray(Wo[cs, :].reshape(NHL, 64, H)).astype(bf),
        })
    return in_maps


def kernel(x, Wq, bq, Wk, bk, Wv, bv, Wo, bo):
    from concourse.bass_utils import run_bass_kernel_spmd

    bo = np.asarray(bo, dtype=np.float32)
    nc = _build()
    in_maps = make_in_maps(x, Wq, bq, Wk, bk, Wv, bv, Wo)
    res = run_bass_kernel_spmd(nc, in_maps, core_ids=list(range(N_CORES)))

    out = np.empty((2, S, H), dtype=np.float32)
    for b in range(2):
        acc = res.results[4 * b]["out"].astype(np.float32)
        for g in range(1, 4):
            acc = acc + res.results[4 * b + g]["out"]
        out[b] = acc + bo
    return out


# revision 29
# speedup vs baseline: 1.0032x; 1.0032x over previous
"""Multi-head self-attention TRN2 Bass kernel (v2: PE-saturated schedule).

Problem: x[2, 2048, 1024], 16 heads x 64 dim, fp32.
Sharding: 8 cores = 2 batches x 4 head-groups (4 heads each).
Each core computes its batch's partial output (its 4 heads through
QKV -> attention -> output projection rows); host sums the 4 partials
per batch and adds bo.

The PE (tensor engine) is the bottleneck (~167us of streamed matmul
at bf16: scores 54.6 + attn@V 54.6 + QKV 41 + out_proj 13.6); the
ACT exp stream is ~142us. v2 therefore schedules for zero PE idle:

  - minimal warmup: only kT m=0 chunk 0 (no bias -- bk is dropped
    entirely: it shifts scores by a per-q constant, softmax-invariant),
    v chunks 0-3, qT m=0 chunk 0. Everything else streams into the
    attention sweep as dependency-placed inline fillers.
  - exp activation table pre-loaded during the initial DMA wait;
    dummy matmuls on a memset tile keep the PE p-state warm while
    the first x chunk lands.
  - scores->AV lag of 2 iterations so the in-order PE queue never
    head-stalls on the ACT stream.
  - normalize (evac + reciprocal + rank-1 PE broadcast) of pair p is
    deferred into pair p+1's early iterations; out_proj of q-chunk qc
    runs inside (qc+1, mt=0)'s sweep.
"""

import numpy as np

S = 2048          # sequence length per batch
H = 1024          # hidden
G = 256           # head-group width (4 heads x 64)
HD = 65           # V' columns per head (64 + ones)
NHL = 4           # heads per core
N_CORES = 8

_CACHE = {}


def _build():
    if "nc" in _CACHE:
        return _CACHE["nc"]

    import concourse.bass as bass
    import concourse.mybir as mybir
    import concourse.tile as tile
    from concourse import bacc
    from concourse.tile_rust import add_dep_helper

    f32 = mybir.dt.float32
    bf16 = mybir.dt.bfloat16
    f8 = mybir.dt.float8e4
    EXP = mybir.ActivationFunctionType.Exp
    DR = mybir.MatmulPerfMode.DoubleRow

    nc = bacc.Bacc("TRN2", target_bir_lowering=False, debug=False,
                   num_devices=N_CORES)

    xt_in = nc.dram_tensor("xt", [4, 128, 8, 512], bf16, kind="ExternalInput")
    wq_in = nc.dram_tensor("wq", [H, G], bf16, kind="ExternalInput")
    wk_in = nc.dram_tensor("wk", [H, G], bf16, kind="ExternalInput")
    wv_in = nc.dram_tensor("wv", [H, G], bf16, kind="ExternalInput")
    bq_in = nc.dram_tensor("bq", [G, 1], f32, kind="ExternalInput")
    bv_in = nc.dram_tensor("bv", [G], f32, kind="ExternalInput")
    wo_in = nc.dram_tensor("wo", [NHL, 64, H], bf16, kind="ExternalInput")
    out_d = nc.dram_tensor("out", [S, H], bf16, kind="ExternalOutput")

    with tile.TileContext(nc) as tc:
        with tc.tile_pool(name="persist", bufs=1) as persist:
            qT = persist.tile([128, 2, S], bf16)     # [qd, m, s]
            kT = persist.tile([128, 2, S], bf16)
            vp = persist.tile([128, 16, NHL, HD], bf16)  # [s-part, st, h, col]
            bq_sb = persist.tile([128, 2, 1], f32)
            bv_f = persist.tile([1, G], f32)
            bv_bf = persist.tile([1, G], bf16)
            bv_bc = persist.tile([128, G], f32)
            wo_pr = persist.tile([128, 2, H], bf16)
            ones64 = persist.tile([1, 64], bf16)
            ones128 = persist.tile([1, 128], bf16)
            warm = persist.tile([128, 512], bf16)
            warm_e = persist.tile([1, 8], f32)
            wq_sb = persist.tile([128, 8, G], bf16)
            wk_sb = persist.tile([128, 8, G], bf16)
            wv_sb = persist.tile([128, 8, G], bf16)
            xTc = [persist.tile([128, 8, 512], bf16, name=f"xT_{jc}")
                   for jc in range(4)]

            with (
                tc.tile_pool(name="at_roll", bufs=2) as at_pool,
                tc.tile_pool(name="outP", bufs=4) as op_pool,
                tc.tile_pool(name="tmpo", bufs=1) as tmpo_pool,
                tc.tile_pool(name="sums", bufs=4) as sums_pool,
                tc.tile_pool(name="osb", bufs=2) as osb_pool,
                tc.tile_pool(name="ps_s", bufs=2, space="PSUM") as ps_s_pool,
                tc.tile_pool(name="ps_av", bufs=2, space="PSUM") as ps_av_pool,
                tc.tile_pool(name="ps_op", bufs=1, space="PSUM") as ps_op_pool,
            ):
                # memsets first so the gpsimd queue isn't blocked
                nc.gpsimd.memset(warm, 0.125)
                nc.gpsimd.memset(vp[:, :, :, 64:65], 1.0)
                nc.gpsimd.memset(ones64, 1.0)
                nc.gpsimd.memset(ones128, 1.0)

                # ---------------- DMAs (two priority chains) ----------------
                # chain A (small): bv -> wk -> wv -> wq -> bq
                dma_bv = nc.sync.dma_start(
                    out=bv_f, in_=bv_in.ap().rearrange("(o g) -> o g", o=1))
                dma_wk = nc.sync.dma_start(
                    out=wk_sb, in_=wk_in.ap().rearrange("(t p) d -> p t d", p=128))
                dma_wv = nc.sync.dma_start(
                    out=wv_sb, in_=wv_in.ap().rearrange("(t p) d -> p t d", p=128))
                dma_wq = nc.sync.dma_start(
                    out=wq_sb, in_=wq_in.ap().rearrange("(t p) d -> p t d", p=128))
                dma_bq = nc.sync.dma_start(
                    out=bq_sb, in_=bq_in.ap().rearrange("(m p) o -> p m o", p=128))
                # chain B (big): xc0 (split in halves so the warmup matmuls
                # can start after 512KB) -> xc1 -> xc2 -> xc3 -> wo
                x0a = nc.sync.dma_start(out=xTc[0][:, 0:4, :],
                                        in_=xt_in.ap()[0][:, 0:4, :])
                x0b = nc.sync.dma_start(out=xTc[0][:, 4:8, :],
                                        in_=xt_in.ap()[0][:, 4:8, :])
                x_dmas = [x0b] + [nc.sync.dma_start(out=xTc[jc],
                                                    in_=xt_in.ap()[jc])
                          for jc in range(1, 4)]
                # Wo as stacked head pairs: [two*64+p, pr, n]
                dma_wo = nc.sync.dma_start(
                    out=wo_pr,
                    in_=wo_in.ap().rearrange("(pr two) p n -> (two p) pr n", two=2))
                add_dep_helper(x0b.ins, x0a.ins, reason="dma order")
                for a, b in [(dma_wk, dma_bv), (dma_wv, dma_wk),
                             (dma_wq, dma_wv), (dma_bq, dma_wq),
                             (x_dmas[1], x_dmas[0]), (x_dmas[2], x_dmas[1]),
                             (x_dmas[3], x_dmas[2]), (dma_wo, x_dmas[3])]:
                    add_dep_helper(a.ins, b.ins, reason="dma order")

                # pre-load the exp activation table (~2.7us) off the
                # critical path
                nc.scalar.activation(out=warm_e, in_=warm[0:1, 0:8], func=EXP)

                def dummy(n=512):
                    ps_d = ps_op_pool.tile([128, 512], f32, tag="dummy",
                                           bufs=1)
                    nc.tensor.matmul(ps_d[:, 0:n], lhsT=warm[:, 0:128],
                                     rhs=warm[:, 0:n], start=True, stop=True)

                for _ in range(4):
                    dummy()
                # bv broadcast along partitions via rank-1 PE outer product
                nc.vector.tensor_copy(bv_bf, bv_f)
                ps_bv = ps_op_pool.tile([128, G], f32, tag="oproj",
                                        name="ps_bv", bufs=1)
                nc.tensor.matmul(ps_bv, lhsT=ones128, rhs=bv_bf,
                                 start=True, stop=True)
                nc.vector.tensor_copy(bv_bc, ps_bv)
                for _ in range(5):
                    dummy()

                # ---------------- QKV building blocks ----------------
                qk_ring = [0]

                def qk_full(w_sb, b_sb, dst, jc, m):
                    """dst[:, m, jc-chunk] = (x @ W)[:, m-half] (+ bias).

                    PSUM comes from the two 1-buf rings (dummy/oproj)
                    alternately so adjacent calls don't WAR-stall."""
                    sl = slice(jc * 512, (jc + 1) * 512)
                    tag = "dummy" if qk_ring[0] == 0 else "oproj"
                    qk_ring[0] ^= 1
                    ps = ps_op_pool.tile([128, 512], f32, tag=tag,
                                         name=f"psqk_{id(w_sb)}_{jc}_{m}",
                                         bufs=1)
                    for ht in range(8):
                        nc.tensor.matmul(
                            ps,
                            lhsT=w_sb[:, ht, m * 128:(m + 1) * 128],
                            rhs=xTc[jc][:, ht, :],
                            start=(ht == 0), stop=(ht == 7))
                    if b_sb is not None:
                        nc.vector.tensor_scalar_add(dst[:, m, sl], ps,
                                                    b_sb[:, m, :])
                    else:
                        nc.vector.tensor_copy(dst[:, m, sl], ps)

                def v_unit(st16):
                    tag = "dummy" if qk_ring[0] == 0 else "oproj"
                    qk_ring[0] ^= 1
                    ps_vt = ps_op_pool.tile([128, 512], f32, tag=tag,
                                            name=f"psv_{st16}", bufs=1)
                    for ht in range(8):
                        nc.tensor.matmul(
                            ps_vt[:, 0:G],
                            lhsT=xTc[st16 // 4][:, ht,
                                                (st16 % 4) * 128:
                                                (st16 % 4 + 1) * 128],
                            rhs=wv_sb[:, ht, :],
                            start=(ht == 0), stop=(ht == 7))
                    nc.vector.tensor_add(
                        vp[:, st16, :, 0:64],
                        ps_vt[:, 0:G].rearrange("p (h d) -> p h d", h=NHL),
                        bv_bc.rearrange("p (h d) -> p h d", h=NHL))

                # warmup: exactly what pair 0 iterations 0-3 need
                qk_full(wk_sb, None, kT, 0, 0)
                for i in range(4):
                    v_unit(i)
                qk_full(wq_sb, bq_sb, qT, 0, 0)

                # ---------------- normalize + out_proj ----------------
                def norm_evac(ps_av, hh, tag):
                    # evacuate PSUM right away to release the bank; MUST be
                    # emitted before the next pair's first AV matmul so the
                    # ring WAR dependency is seen
                    uout = tmpo_pool.tile([HD, 512], f32, tag="uout",
                                          name=f"uo_{tag}_{hh}", bufs=4)
                    nc.vector.tensor_copy(uout, ps_av)
                    return uout

                def norm_recip(uout, hh, tag):
                    # DVE chain: sums copy -> fast reciprocal -> bf16 cast
                    sums = sums_pool.tile([1, 512], f32, tag="sums",
                                          name=f"sm_{tag}_{hh}")
                    nc.vector.tensor_copy(sums, uout[64:65, :])
                    recip = sums_pool.tile([1, 512], f32, tag="recip",
                                           name=f"rc_{tag}_{hh}")
                    nc.vector.reciprocal_approx_fast(out=recip, in_=sums)
                    recip_bf = sums_pool.tile([1, 512], bf16, tag="recipb",
                                              name=f"rcb_{tag}_{hh}")
                    nc.vector.tensor_copy(recip_bf, recip)
                    return recip_bf

                def norm_fin(outP, uout, recip_bf, hh, tag, tail=False):
                    if tail:
                        # at the kernel tail the PE is idle and the gpsimd
                        # broadcast's ~1us latency is serial: use the rank-1
                        # PE outer product instead
                        rbc_ps = ps_op_pool.tile([64, 512], f32, tag="dummy",
                                                 name=f"rbp_{tag}_{hh}",
                                                 bufs=1)
                        nc.tensor.matmul(rbc_ps, lhsT=ones64, rhs=recip_bf,
                                         start=True, stop=True)
                        nc.vector.tensor_mul(
                            outP[hh * 64:hh * 64 + 64, :], uout[0:64, :],
                            rbc_ps)
                        return
                    # broadcast along partitions on the idle GPSIMD engine
                    # (keeps the PE out of the normalize chain entirely)
                    rbc = sums_pool.tile([64, 512], bf16, tag="rbc",
                                         name=f"rb_{tag}_{hh}")
                    nc.gpsimd.partition_broadcast(rbc, recip_bf)
                    nc.vector.tensor_mul(
                        outP[hh * 64:hh * 64 + 64, :], uout[0:64, :], rbc)

                def oproj_unit(qc, outPs, qt, tail=False):
                    # out_proj for one q-tile (K=128 stacked pairs); the two
                    # ncx halves alternate the 1-buf rings so ncx1's matmuls
                    # don't WAR-stall on ncx0's evacuation; at the kernel
                    # tail the freed score slots double-buffer it
                    osb = osb_pool.tile([128, H], bf16, tag="osb",
                                        name=f"osb_{qc}_{qt}")
                    for ncx in range(2):
                        if tail:
                            ps_op = ps_s_pool.tile(
                                [128, 2, 512], f32, tag="s",
                                name=f"psot_{qc}_{qt}_{ncx}")[:, 0, :]
                        else:
                            ps_op = ps_op_pool.tile(
                                [128, 512], f32,
                                tag="oproj" if ncx == 0 else "dummy",
                                name=f"pso_{qc}_{qt}_{ncx}", bufs=1)
                        for pr in range(2):
                            nc.tensor.matmul(
                                ps_op,
                                lhsT=outPs[pr][:, qt * 128:(qt + 1) * 128],
                                rhs=wo_pr[:, pr, ncx * 512:(ncx + 1) * 512],
                                start=(pr == 0), stop=(pr == 1))
                        if tail and ncx == 1:
                            # ACT is idle after the last exp -- split the
                            # tail evacuations across ACT and DVE
                            nc.scalar.copy(
                                osb[:, ncx * 512:(ncx + 1) * 512], ps_op)
                        else:
                            nc.vector.tensor_copy(
                                osb[:, ncx * 512:(ncx + 1) * 512], ps_op)
                    nc.sync.dma_start(
                        out=out_d.ap()[qc * 512 + qt * 128:
                                       qc * 512 + (qt + 1) * 128, :],
                        in_=osb)

                # ---------------- attention sweep ----------------
                pending_norm = None   # (outP, ps_avs, [uouts], [recips], tag)
                prev_oproj = None     # (qc, outPs) awaiting out_proj
                pair_idx = 0
                for qc in range(4):  # q-chunks of 512
                    qsl = slice(qc * 512, (qc + 1) * 512)
                    outPs = {}
                    # qc3 runs mt1 first so the kernel tail only carries
                    # mt0's normalize
                    for mt in ((1, 0) if qc == 3 else (0, 1)):
                        tag = f"{qc}_{mt}"
                        attnT = at_pool.tile([128, 2, 4, 512], bf16,
                                             tag="at", name=f"at_{tag}")
                        ps_avs = [ps_av_pool.tile([HD, 512], f32, tag="av",
                                                  name=f"av_{tag}_{hh}")
                                  for hh in range(2)]

                        def av_mm(kt, ps_avs=ps_avs, attnT=attnT, mt=mt):
                            for hh in range(2):
                                nc.tensor.matmul(
                                    ps_avs[hh],
                                    lhsT=vp[:, kt, 2 * mt + hh, :],
                                    rhs=attnT[:, hh, kt % 4, :],
                                    start=(kt == 0), stop=(kt == 15))

                        for kt in range(16):
                            # deferred normalize of the previous pair FIRST
                            # so its DVE ops aren't head-blocked behind the
                            # inline qk evacuation: evacuations at kt 0/1
                            # (before av_mm(0) below), DVE recip chains at
                            # kt 2/3, bcast+mul at 4/6
                            if pending_norm is not None:
                                pP, pavs, puo, prc, ptag = pending_norm
                                if kt in (0, 1):
                                    puo.append(norm_evac(pavs[kt], kt, ptag))
                                elif kt in (2, 3):
                                    prc.append(norm_recip(puo[kt - 2],
                                                          kt - 2, ptag))
                                elif kt == 4:
                                    norm_fin(pP, puo[0], prc[0], 0, ptag)
                                elif kt == 6:
                                    norm_fin(pP, puo[1], prc[1], 1, ptag)
                                    pending_norm = None
                            # inline QKV fillers, placed just before need
                            if pair_idx == 0:
                                if kt % 4 == 0 and kt > 0:
                                    qk_full(wk_sb, None, kT, kt // 4, 0)
                                if 2 <= kt <= 13:
                                    v_unit(kt + 2)
                            elif pair_idx == 1 and kt % 4 == 0:
                                if kt == 0:
                                    qk_full(wq_sb, bq_sb, qT, 0, 1)
                                qk_full(wk_sb, None, kT, kt // 4, 1)
                            elif pair_idx >= 2 and kt == 0:
                                qk_full(wq_sb, bq_sb, qT, qc, mt)
                            # deferred out_proj of the previous q-chunk
                            if prev_oproj is not None and kt in (12, 14):
                                pq, pouts = prev_oproj
                                qt0 = 0 if kt == 12 else 2
                                oproj_unit(pq, pouts, qt0)
                                oproj_unit(pq, pouts, qt0 + 1)
                                if kt == 14:
                                    prev_oproj = None
                            # scores (transposed): S^T[k,q] = kT.T @ qT
                            ps_s = ps_s_pool.tile([128, 2, 512], f32, tag="s")
                            for hh in range(2):
                                nc.tensor.matmul(
                                    ps_s[:, hh, :],
                                    lhsT=kT[hh * 64:hh * 64 + 64, mt,
                                            kt * 128:(kt + 1) * 128],
                                    rhs=qT[hh * 64:hh * 64 + 64, mt, qsl],
                                    start=True, stop=True)
                            nc.scalar.activation(
                                out=attnT[:, :, kt % 4, :], in_=ps_s, func=EXP)
                            if kt >= 2:
                                av_mm(kt - 2)
                        av_mm(14)
                        av_mm(15)

                        outP = op_pool.tile([128, 512], bf16, tag="outP",
                                            name=f"outP_{tag}")
                        pending_norm = (outP, ps_avs, [], [], tag)
                        outPs[mt] = outP
                        pair_idx += 1
                    prev_oproj = (qc, [outPs[0], outPs[1]])

                # tail: last pair's normalize, then final out_proj; the
                # uout evacuations ride the idle ACT engine
                pP, pavs, puo, prc, ptag = pending_norm
                for hh in range(2):
                    uout = tmpo_pool.tile([HD, 512], f32, tag="uout",
                                          name=f"uo_{ptag}_{hh}", bufs=4)
                    nc.scalar.copy(uout, pavs[hh])
                    puo.append(uout)
                for hh in range(2):
                    prc.append(norm_recip(puo[hh], hh, ptag))
                for hh in range(2):
                    norm_fin(pP, puo[hh], prc[hh], hh, ptag, tail=True)
                pq, pouts = prev_oproj
                for qt in range(4):
                    oproj_unit(pq, pouts, qt, tail=True)

    nc.compile()
    _CACHE["nc"] = nc
    return nc


def make_in_maps(x, Wq, bq, Wk, bk, Wv, bv, Wo):
    import ml_dtypes
    bf = ml_dtypes.bfloat16

    x = np.asarray(x, dtype=np.float32)
    Wq = np.asarray(Wq, dtype=np.float32)
    bq = np.asarray(bq, dtype=np.float32)
    Wk = np.asarray(Wk, dtype=np.float32)
    Wv = np.asarray(Wv, dtype=np.float32)
    bv = np.asarray(bv, dtype=np.float32)
    Wo = np.asarray(Wo, dtype=np.float32)

    scale = np.float32(1.0 / 8.0)  # 1/sqrt(64)

    in_maps = []
    for core in range(N_CORES):
        b = core // 4
        g = core % 4
        cs = slice(g * G, (g + 1) * G)
        in_maps.append({
            "xt": np.ascontiguousarray(
                x[b].reshape(4, 512, 8, 128).transpose(0, 3, 2, 1)).astype(bf),
            "wq": np.ascontiguousarray(Wq[:, cs] * scale).astype(bf),
            "wk": np.ascontiguousarray(Wk[:, cs]).astype(bf),
            "wv": np.ascontiguousarray(Wv[:, cs]).astype(bf),
            "bq": np.ascontiguousarray((bq[cs] * scale).reshape(G, 1)),
            "bv": np.ascontiguousarray(bv[cs]),
            "wo": np.ascontiguousarray(Wo[cs, :].reshape(NHL, 64, H)).astype(bf),
        })
    return in_maps


def kernel(x, Wq, bq, Wk, bk, Wv, bv, Wo, bo):
    from concourse.bass_utils import run_bass_kernel_spmd

    bo = np.asarray(bo, dtype=np.float32)
    nc = _build()
    in_maps = make_in_maps(x, Wq, bq, Wk, bk, Wv, bv, Wo)
    res = run_bass_kernel_spmd(nc, in_maps, core_ids=list(range(N_CORES)))

    out = np.empty((2, S, H), dtype=np.float32)
    for b in range(2):
        acc = res.results[4 * b]["out"].astype(np.float32)
        for g in range(1, 4):
            acc = acc + res.results[4 * b + g]["out"]
        out[b] = acc + bo
    return out
